# revision 1
# baseline (speedup 1.0000x reference)
"""Contrastive loss (soft-target NT-Xent style) on 8 Trainium2 NeuronCores.

Math (matches the reference):
    e = x / max(||x||, eps)              row-normalized embeddings
    sim = e @ e.T / T                    T = 0.1
    logz_i = logsumexp_{j != i} sim[i, j]
    row_loss_i = sum_{j: l_j == l_i, j != i} (logz_i - sim[i, j])
    loss = sum_i row_loss_i / N

Device decomposition (per core; core c's inputs are rotated by c*1024 along
the sample axis so all 8 cores run the identical program on "local rows
0..1023"):
    sumexp_i = sum_j exp(10*g_ij - 10) - 1       (diag g_ii ~= 1 exactly)
    logz_i   = 10 + ln(sumexp_i)
    d_i      = e_i . S_{l_i},  S_k = sum_{j: l_j = k} e_j   (via 2 matmuls)
    row_loss_i = (C_{l_i} - 1) * logz_i - 10 * (d_i - 1)
               = sum_k OH[i,k] * (C_k * logzf_i - 10*T[i,k]) - lnres_i
    (logzf = 10 + lnres, lnres = ln(sumexp))
Each core returns the scalar sum of its 1024 row losses; host sums and /N.
"""

import numpy as np

import concourse.bass as bass
import concourse.bacc as bacc
import concourse.tile as tile
from concourse import mybir
from concourse.masks import make_identity

N = 8192
D = 128
NCLASS = 100
NCORES = 8
ROWS = N // NCORES  # rows per core (1024)
MT = ROWS // 128  # m-tiles per core (8)
CH = N // 128  # 128-row chunks (64)
NS = 8  # column slices (each 1024 wide)
SW = N // NS  # slice width (1024)
NG = 4  # psum groups per m-tile (each 2048 wide)
TEMP_INV = 10.0  # 1 / temperature

F32 = mybir.dt.float32
BF16 = mybir.dt.bfloat16


def build_nc(loop_k: int = 1, stage: int = 4, accum: bool = True, erm_act: bool = False, early_tail: bool = True, trp_bufs: int = 2, ed_bufs: int = 2):
    """Build the per-core Bass program. loop_k > 1 wraps the whole body in a
    hardware loop (timing amortization only). stage < 4 builds a prefix."""
    nc = bacc.Bacc("TRN2", target_bir_lowering=False, debug=False)

    xt_d = nc.dram_tensor("xt", [128, N], BF16, kind="ExternalInput")
    ohb_d = nc.dram_tensor("ohb", [128, CH * NCLASS], BF16, kind="ExternalInput")
    oh8_d = nc.dram_tensor("oh8", [128, MT * NCLASS], F32, kind="ExternalInput")
    cb_d = nc.dram_tensor("cb", [128, NCLASS], F32, kind="ExternalInput")
    out_d = nc.dram_tensor("out", [1, 1], F32, kind="ExternalOutput")
    rn_scr = nc.dram_tensor("rn_scr", [N], F32)  # internal scratch

    with tile.TileContext(nc) as tc:
        with (
            tc.tile_pool(name="persist", bufs=1) as persist,
            tc.tile_pool(name="edum", bufs=ed_bufs) as edum_pool,
        ):
            # ---- persistent SBUF tiles ----
            xtb_s = [persist.tile([128, SW], BF16, tag=f"xtb{i}", name=f"xtb{i}") for i in range(NS)]
            sqt_s = [persist.tile([128, SW], BF16, tag=f"sq{i}", name=f"sq{i}") for i in range(NS)]
            etn_s = [persist.tile([128, SW], BF16, tag=f"etn{i}", name=f"etn{i}") for i in range(NS)]
            rnb_s = [persist.tile([128, SW], F32, tag=f"rnb{i}", name=f"rnb{i}") for i in range(NS)]
            erm_g = [
                persist.tile([128, MT, 128], BF16, tag=f"erm{g}", name=f"erm{g}") for g in range(NS)
            ]
            ohb = persist.tile([128, CH, NCLASS], BF16, tag="ohb")
            oh8 = persist.tile([128, MT, NCLASS], F32, tag="oh8")
            cb = persist.tile([128, NCLASS], F32, tag="cb")
            ident = persist.tile([128, 128], BF16, tag="ident")
            identf = persist.tile([128, 128], F32, tag="identf")
            onesb = persist.tile([128, 1], BF16, tag="onesb")
            n2 = persist.tile([128, CH], F32, tag="n2")
            lnn2 = persist.tile([128, CH], F32, tag="lnn2")
            rnorm = persist.tile([128, CH], F32, tag="rnorm")
            rnT = persist.tile([64, 128], F32, tag="rnT")
            stsb = persist.tile([128, NCLASS], BF16, tag="stsb")
            t10 = persist.tile([128, MT, NCLASS], F32, tag="t10")
            expacc = persist.tile([128, MT * NG], F32, tag="expacc")
            sum4 = persist.tile([128, MT], F32, tag="sum4")
            lnres = persist.tile([128, MT], F32, tag="lnres")
            logzf = persist.tile([128, MT], F32, tag="logzf")
            rl = persist.tile([128, MT], F32, tag="rl")
            am = persist.tile([128, MT], F32, tag="am")
            bm = persist.tile([128, MT], F32, tag="bm")
            rlrow = persist.tile([128, 1], F32, tag="rlrow")
            ones = persist.tile([128, 1], F32, tag="ones")
            u0 = persist.tile([128, NCLASS], F32, tag="u0")
            u1 = persist.tile([128, NCLASS], F32, tag="u1")
            outsb = persist.tile([1, 1], F32, tag="outsb")
            bneg10 = persist.tile([128, 1], F32, tag="bneg10")
            bneg1 = persist.tile([128, 1], F32, tag="bneg1")

            make_identity(nc, ident[:])
            make_identity(nc, identf[:])
            nc.vector.memset(onesb[:], 1.0)
            nc.vector.memset(ones[:], 1.0)
            nc.vector.memset(bneg10[:], -TEMP_INV)
            nc.vector.memset(bneg1[:], -1.0)

            ohbv = ohb_d.rearrange("p (c k) -> p c k", k=NCLASS)
            oh8v = oh8_d.rearrange("p (m k) -> p m k", k=NCLASS)

            def finish(src):
                nc.vector.tensor_reduce(
                    out=rlrow[:],
                    in_=src,
                    axis=mybir.AxisListType.X,
                    op=mybir.AluOpType.add,
                )
                with tc.tile_pool(name="fpsum", bufs=1, space="PSUM") as fpsum:
                    fin = fpsum.tile([1, 1], F32, tag="fin")
                    nc.tensor.matmul(fin[:], rlrow[:], ones[:], start=True, stop=True)
                    nc.vector.tensor_copy(outsb[:], fin[:])
                nc.sync.dma_start(out_d[:], outsb[:])

            def body():
                # ---- input DMA ----
                for i in range(NS):
                    nc.sync.dma_start(xtb_s[i][:], xt_d[:, i * SW : (i + 1) * SW])
                for g in range(4):
                    s = slice(g * (CH // 4), (g + 1) * (CH // 4))
                    nc.sync.dma_start(ohb[:, s, :], ohbv[:, s, :])
                nc.sync.dma_start(oh8[:], oh8v[:])
                nc.sync.dma_start(cb[:], cb_d[:])

                # ---- pass A: squares -> column-sum matmuls -> rsqrt ->
                # transpose + DRAM-broadcast round trip ----
                with tc.tile_pool(name="prep", bufs=1, space="PSUM") as prep:
                    for i in range(NS):
                        nc.vector.tensor_mul(
                            sqt_s[i][:], xtb_s[i][:], xtb_s[i][:]
                        )
                    n2ps = prep.tile([128, CH], F32, tag="n2ps")
                    for c in range(CH):
                        i, j = divmod(c, MT)
                        nc.tensor.matmul(
                            n2ps[:, c : c + 1],
                            sqt_s[i][:, j * 128 : (j + 1) * 128],
                            onesb[:],
                            start=True,
                            stop=True,
                        )
                    # rnorm = exp(-0.5 * ln(n2)), straight from PSUM
                    nc.scalar.activation(
                        lnn2[:], n2ps[:], mybir.ActivationFunctionType.Ln
                    )
                    nc.scalar.activation(
                        rnorm[:],
                        lnn2[:],
                        mybir.ActivationFunctionType.Exp,
                        scale=-0.5,
                    )
                    rntp = prep.tile([64, 128], F32, tag="rnt")
                    nc.tensor.transpose(rntp[:], rnorm[:], identf[:])
                    nc.vector.tensor_copy(rnT[:], rntp[:])
                    nc.sync.dma_start(rn_scr[:], rnT[:])
                    for i in range(NS):
                        sl = rn_scr[i * SW : (i + 1) * SW]
                        bcast = bass.AP(
                            tensor=sl.tensor,
                            offset=sl.offset,
                            ap=[[0, 128]] + list(sl.ap),
                        )
                        nc.sync.dma_start(rnb_s[i][:], bcast)
                    if stage == 1:
                        finish(rnb_s[0][:, :64])
                        return

                    # ---- pass B (per slice): normalized e^T -> row-major e ->
                    # ST accumulation.  PSUM->SBUF copies go on the (idle)
                    # scalar engine so the vector engine only does the muls.
                    st_ps = prep.tile([128, NCLASS], F32, tag="st")
                    for i in range(NS):
                        nc.vector.tensor_mul(
                            etn_s[i][:], xtb_s[i][:], rnb_s[i][:]
                        )
                        trp = prep.tile([128, MT, 128], BF16, tag="trp", bufs=trp_bufs)
                        for j in range(MT):
                            nc.tensor.transpose(
                                trp[:, j, :],
                                etn_s[i][:, j * 128 : (j + 1) * 128],
                                ident[:],
                            )
                        if erm_act:
                            nc.scalar.copy(erm_g[i][:], trp[:])
                        else:
                            nc.vector.tensor_copy(erm_g[i][:], trp[:])
                        for j in range(MT):
                            c = i * MT + j
                            nc.tensor.matmul(
                                st_ps[:],
                                erm_g[i][:, j, :],
                                ohb[:, c, :],
                                start=(c == 0),
                                stop=(c == CH - 1),
                            )
                    nc.vector.tensor_copy(stsb[:], st_ps[:])
                    for m in range(MT):
                        tm_ps = prep.tile([128, NCLASS], F32, tag="tm", bufs=2)
                        nc.tensor.matmul(
                            tm_ps[:],
                            etn_s[0][:, m * 128 : (m + 1) * 128],
                            stsb[:],
                            start=True,
                            stop=True,
                        )
                        nc.vector.tensor_scalar_mul(t10[:, m, :], tm_ps[:], TEMP_INV)

                if stage == 2:
                    finish(t10[:, :, :].rearrange("p a b -> p (a b)"))
                    return

                # ---- main loop: sim blocks + fused exp/accumulate ----
                with tc.tile_pool(name="mpsum", bufs=2, space="PSUM") as mpsum:
                    for m in range(MT):
                        lhsT = etn_s[0][:, m * 128 : (m + 1) * 128]
                        for g in range(NG):
                            ps = mpsum.tile([128, 2048], F32, tag="ps")
                            for q in range(4):
                                n0 = (g * 4 + q) * 512
                                nc.tensor.matmul(
                                    ps[:, q * 512 : (q + 1) * 512],
                                    lhsT,
                                    etn_s[n0 // SW][:, n0 % SW : n0 % SW + 512],
                                    start=True,
                                    stop=True,
                                )
                            ed = edum_pool.tile([128, 2048], BF16, tag="ed")
                            if accum:
                                nc.scalar.activation(
                                    ed[:],
                                    ps[:],
                                    mybir.ActivationFunctionType.Exp,
                                    bias=bneg10[:],
                                    scale=TEMP_INV,
                                    accum_out=expacc[
                                        :, m * NG + g : m * NG + g + 1
                                    ],
                                )
                            else:
                                nc.scalar.activation(
                                    ed[:],
                                    ps[:],
                                    mybir.ActivationFunctionType.Exp,
                                    bias=bneg10[:],
                                    scale=TEMP_INV,
                                )
                                nc.vector.tensor_reduce(
                                    out=expacc[:, m * NG + g : m * NG + g + 1],
                                    in_=ed[:],
                                    axis=mybir.AxisListType.X,
                                    op=mybir.AluOpType.add,
                                )
                        nc.vector.tensor_reduce(
                            out=sum4[:, m : m + 1],
                            in_=expacc[:, m * NG : (m + 1) * NG],
                            axis=mybir.AxisListType.X,
                            op=mybir.AluOpType.add,
                        )
                        if early_tail:
                            nc.vector.scalar_tensor_tensor(
                                out=u0[:],
                                in0=oh8[:, m, :],
                                scalar=1.0,
                                in1=cb[:],
                                op0=mybir.AluOpType.mult,
                                op1=mybir.AluOpType.mult,
                                accum_out=am[:, m : m + 1],
                            )
                            nc.vector.scalar_tensor_tensor(
                                out=u1[:],
                                in0=oh8[:, m, :],
                                scalar=1.0,
                                in1=t10[:, m, :],
                                op0=mybir.AluOpType.mult,
                                op1=mybir.AluOpType.mult,
                                accum_out=bm[:, m : m + 1],
                            )

                if stage == 3:
                    finish(sum4[:])
                    return

                # ---- row losses ----
                nc.scalar.activation(
                    lnres[:], sum4[:], mybir.ActivationFunctionType.Ln, bias=bneg1[:]
                )
                nc.vector.tensor_scalar_add(logzf[:], lnres[:], TEMP_INV)
                if early_tail:
                    # row_loss = am*logzf - bm - lnres
                    nc.vector.tensor_mul(rl[:], am[:], logzf[:])
                    nc.vector.tensor_sub(rl[:], rl[:], bm[:])
                    nc.vector.tensor_sub(rl[:], rl[:], lnres[:])
                else:
                    for m in range(MT):
                        nc.vector.scalar_tensor_tensor(
                            out=u0[:],
                            in0=cb[:],
                            scalar=logzf[:, m : m + 1],
                            in1=t10[:, m, :],
                            op0=mybir.AluOpType.mult,
                            op1=mybir.AluOpType.subtract,
                        )
                        nc.vector.scalar_tensor_tensor(
                            out=u1[:],
                            in0=u0[:],
                            scalar=1.0,
                            in1=oh8[:, m, :],
                            op0=mybir.AluOpType.mult,
                            op1=mybir.AluOpType.mult,
                            accum_out=rl[:, m : m + 1],
                        )
                    nc.vector.tensor_sub(rl[:], rl[:], lnres[:])
                finish(rl[:])

            if loop_k == 1:
                body()
            else:
                with tc.For_i(0, loop_k, 1):
                    body()

    nc.compile()
    return nc


def prepare_inputs(embeddings: np.ndarray, labels: np.ndarray):
    """Host-side shard prep: per-core rotated views + label one-hots."""
    import ml_dtypes

    x = np.ascontiguousarray(np.asarray(embeddings, dtype=np.float32))
    lab = np.asarray(labels).astype(np.int64).ravel()
    counts = np.bincount(lab, minlength=NCLASS).astype(np.float32)
    cb_host = np.ascontiguousarray(np.broadcast_to(counts[None, :], (128, NCLASS)))
    in_maps = []
    for c in range(NCORES):
        perm = np.roll(np.arange(N), -c * ROWS)
        xr = x[perm]  # [N, D] rotated
        xt_host = np.ascontiguousarray(xr.T.astype(ml_dtypes.bfloat16))  # [128, N]
        oh = (lab[perm, None] == np.arange(NCLASS)[None, :]).astype(np.float32)
        # [N, K] -> [CH, 128, K] -> [128, CH, K]
        oh_pck = oh.reshape(CH, 128, NCLASS).transpose(1, 0, 2)
        ohb_host = np.ascontiguousarray(
            oh_pck.reshape(128, -1).astype(ml_dtypes.bfloat16)
        )
        oh8_host = np.ascontiguousarray(oh_pck[:, :MT, :].reshape(128, -1))
        in_maps.append(
            {"xt": xt_host, "ohb": ohb_host, "oh8": oh8_host, "cb": cb_host}
        )
    return in_maps


_NC_CACHE = {}


def kernel(embeddings: np.ndarray, labels: np.ndarray) -> np.ndarray:
    from concourse.bass_utils import run_bass_kernel_spmd

    nc = _NC_CACHE.get("nc")
    if nc is None:
        nc = _NC_CACHE["nc"] = build_nc(loop_k=1)
    in_maps = prepare_inputs(embeddings, labels)
    res = run_bass_kernel_spmd(nc, in_maps, list(range(NCORES)))
    total = sum(float(r["out"][0, 0]) for r in res.results)
    return np.asarray(total / N, dtype=np.float32)

